# revision 74
# baseline (speedup 1.0000x reference)
"""Longformer-style windowed self-attention for TRN2, 8-core SPMD.

Sharding: 24 (batch, head) pairs -> 3 heads per core (core c gets batch c//4,
heads (c%4)*3 .. +3). Each core computes QKV projections for its head slice,
windowed attention (query blocks of 128, window +-256), and writes its
[4096, 192] output channel slice. Host gathers slices into [2, 4096, 768].

QKV projections run as fp8e4m3 DoubleRow matmuls (K=256 per step, 0.5
cycles/row), error-compensated with hi/lo splits of both operands
(h_hi*w_hi + h_lo*w_hi + h_hi*w_lo), which keeps fp8 quantization error
below bf16 rounding. Attention matmuls run in bf16. Scores are computed
transposed ([key, query]) per 128-key chunk so the softmax probabilities can
feed the PV matmul as the stationary operand, producing output directly in
[query, head*65] layout with a QKS-valued ones-column per head giving the
softmax denominator (cancelling the fp8 weight prescale). Band masking is
one strided multiplicative bf16 tensor-tensor on DVE; exp runs on Act with
the fp8 descale folded into its scale argument; PSUM evacuation on DVE. The
final softmax division (out[..h*65+64] / out[h*65+64]) happens on the host.

Blocks are processed in descending order with projection pieces emitted
lazily at their consumer deadlines so the PE stays fed during the Act-bound
stretch; dummy warmup matmuls ramp the PE clock while input DMAs land.
"""

import sys

for _p in ("/opt/trn_rl_repo", "/opt/pypackages"):
    if _p not in sys.path:
        sys.path.append(_p)

import numpy as np
from contextlib import ExitStack

import concourse.bass as bass
import concourse.bacc as bacc
import concourse.mybir as mybir
import concourse.tile as tile
from concourse.bass_utils import run_bass_kernel_spmd

F32 = mybir.dt.float32
BF16 = mybir.dt.bfloat16
FP8 = mybir.dt.float8e4
DR = mybir.MatmulPerfMode.DoubleRow
EXP = mybir.ActivationFunctionType.Exp
QKS = 64.0              # fp8 q/k weight prescale; exp() descales by 1/(QKS*QKS*8)

B, S, D = 2, 4096, 768
H, DH = 12, 64
QB = 128                # query block size
NQB = S // QB           # 32 query blocks
NKC = S // 128          # 32 key chunks of 128
HPC = 3                 # heads per core
N_CORES = 8
NT = 8                  # projection s-tiles of 512


def build_program(has_bias, has_kmask):
    nc = bacc.Bacc("TRN2", target_bir_lowering=False, debug=False,
                   num_devices=N_CORES)
    hs8_d = nc.declare_dram_parameter("hs8", [D, S], FP8, isOutput=False)
    hs8l_d = nc.declare_dram_parameter("hs8l", [D, S], FP8, isOutput=False)
    wqk_d = nc.declare_dram_parameter("wqk8", [D, 384], FP8, isOutput=False)
    wqkl_d = nc.declare_dram_parameter("wqk8l", [D, 384], FP8, isOutput=False)
    wv_d = nc.declare_dram_parameter("wv8", [D, 192], FP8, isOutput=False)
    wvl_d = nc.declare_dram_parameter("wv8l", [D, 192], FP8, isOutput=False)
    m3_d = nc.declare_dram_parameter("mask3", [128, 768], BF16, isOutput=False)
    if has_bias:
        bqk_d = nc.declare_dram_parameter("bqk", [1, 384], BF16, isOutput=False)
        bv_d = nc.declare_dram_parameter("bv", [1, 192], BF16, isOutput=False)
    if has_kmask:
        kpad_d = nc.declare_dram_parameter("kpad", [128, NKC], F32, isOutput=False)
    out_d = nc.declare_dram_parameter("out", [S, 195], F32, isOutput=True)

    with tile.TileContext(nc) as tc, ExitStack() as ctx:
        const_p = ctx.enter_context(tc.tile_pool(name="const", bufs=1))
        hst_p = ctx.enter_context(tc.tile_pool(name="hst", bufs=1))
        qkt_p = ctx.enter_context(tc.tile_pool(name="qkt", bufs=1))
        vall_p = ctx.enter_context(tc.tile_pool(name="vall", bufs=1))
        pt_p = ctx.enter_context(tc.tile_pool(name="pt", bufs=25))
        wk_p = ctx.enter_context(tc.tile_pool(name="wk", bufs=11))
        sab_p = ctx.enter_context(tc.tile_pool(name="sab", bufs=1, space="PSUM"))
        sc_p = ctx.enter_context(tc.tile_pool(name="sc", bufs=1, space="PSUM"))
        prj_p = ctx.enter_context(tc.tile_pool(name="prj", bufs=3, space="PSUM"))

        # ---- input DMAs, finest-first so proj of tile 7 starts early ----
        hst8 = hst_p.tile([128, 6, S], FP8)
        hst8l = hst_p.tile([128, 6, S], FP8)
        hs8_r = hs8_d[:].rearrange("(c p) s -> p c s", p=128)
        hs8l_r = hs8l_d[:].rearrange("(c p) s -> p c s", p=128)
        wqk_sb = const_p.tile([128, 6, 384], FP8)
        wqkl_sb = const_p.tile([128, 6, 384], FP8)
        wqk_r = wqk_d[:].rearrange("(c p) n -> p c n", p=128)
        nc.sync.dma_start(wqk_sb[:], wqk_r)

        def dma_hst_tile(t):
            sl = slice(512 * t, 512 * t + 512)
            nc.sync.dma_start(hst8[:, :, sl], hs8_r[:, :, sl])
            nc.sync.dma_start(hst8l[:, :, sl], hs8l_r[:, :, sl])

        nc.sync.dma_start(hst8[:, :, 512 * 7 :], hs8_r[:, :, 512 * 7 :])
        nc.sync.dma_start(wqkl_sb[:], wqkl_d[:].rearrange("(c p) n -> p c n", p=128))
        nc.sync.dma_start(hst8l[:, :, 512 * 7 :], hs8l_r[:, :, 512 * 7 :])
        wv_sb = const_p.tile([128, 6, 192], FP8)
        wvl_sb = const_p.tile([128, 6, 192], FP8)
        m3_sb = const_p.tile([128, 2, 3, 128], BF16)

        def deferred_const_dmas():
            # queued behind the first qk lump's kt_c shift so block 31's
            # critical chain is not delayed by these later-needed transfers
            nc.sync.dma_start(wv_sb[:], wv_d[:].rearrange("(c p) n -> p c n", p=128))
            nc.sync.dma_start(wvl_sb[:], wvl_d[:].rearrange("(c p) n -> p c n", p=128))
            nc.sync.dma_start(
                m3_sb[:], m3_d[:].rearrange("p (s h c) -> p s h c", s=2, h=3)
            )
            dma_hst_tile(6)
        if has_bias:
            bqk_sb = const_p.tile([1, 384], BF16)
            nc.sync.dma_start(bqk_sb[:], bqk_d[:, :])
            bv_sb = const_p.tile([1, 192], BF16)
            nc.sync.dma_start(bv_sb[:], bv_d[:, :])
            ones_sb = const_p.tile([1, 512], BF16)
            nc.vector.memset(ones_sb[:], 1.0)
        if has_kmask:
            kpad_sb = const_p.tile([128, NKC], F32)
            nc.sync.dma_start(kpad_sb[:], kpad_d[:, :])

        # PE warmup: dependency-free dummy matmuls ramp the PE to full
        # clock while the first input DMAs are still in flight.
        warm = const_p.tile([128, 512], BF16)
        nc.vector.memset(warm[:], 0.0)
        for i in range(34):
            wp = prj_p.tile([128, 512], F32, space="PSUM", tag="prj")
            w = 512 if i < 5 else 128
            nc.tensor.matmul(wp[:, 0:w], warm[:, 0:128], warm[:, 0:w],
                             start=True, stop=True)

        # qT/kT for head pair (A,B): A on partitions 0:64, B on 64:128
        qt_ab = qkt_p.tile([128, S], BF16)
        kt_ab = qkt_p.tile([128, S], BF16)
        # head C: [qC; kC] packed on partitions 0:64 / 64:128; kt_c is the
        # partition-shifted copy so it can be a (0,0)-tile stationary operand
        qkt_c = qkt_p.tile([128, S], BF16)
        kt_c = qkt_p.tile([64, S], BF16)
        # v in [key, dh] layout: [128, key-chunk, (vA|1|vB|1|vC|1)]
        vall = vall_p.tile([128, NKC, 195], BF16)
        ones_cols = vall[:].rearrange("p m (h x) -> p m h x", h=3)[:, :, :, 64:65]
        nc.gpsimd.memset(ones_cols, QKS)

        def emit_proj_qk_j(t, j):
            s0 = 512 * t
            hs8 = hst8[:, :, s0 : s0 + 512]
            hs8l = hst8l[:, :, s0 : s0 + 512]
            # q/k projections: 3 pair-matmuls of M=128 in fp8 DoubleRow
            # (K=256 per step), error-compensated as
            # h_hi*w_hi + h_lo*w_hi + h_hi*w_lo; weights prescaled by QKS,
            # descaled in the exp
            if True:
                pp = prj_p.tile([128, 512], F32, space="PSUM", tag="prj")
                terms = ((wqk_sb, hs8), (wqkl_sb, hs8), (wqk_sb, hs8l))
                for ti, (wt, ht) in enumerate(terms):
                    for c in range(0, 6, 2):
                        nc.tensor.matmul(
                            pp[:],
                            (wt[:, c : c + 2, 128 * j : 128 * j + 128]),
                            (ht[:, c : c + 2, :]),
                            start=(ti == 0 and c == 0),
                            stop=(ti == 2 and c == 4 and not has_bias),
                            perf_mode=DR,
                        )
                if has_bias:
                    nc.tensor.matmul(
                        pp[:],
                        (bqk_sb[0:1, 128 * j : 128 * j + 128]),
                        (ones_sb[0:1, :]),
                        start=False,
                        stop=True,
                    )
                if j == 0:
                    nc.vector.tensor_copy(qt_ab[:, s0 : s0 + 512], pp[:])
                elif j == 1:
                    nc.vector.tensor_copy(kt_ab[:, s0 : s0 + 512], pp[:])
                else:
                    nc.vector.tensor_copy(qkt_c[:, s0 : s0 + 512], pp[:])
                    nc.sync.dma_start(
                        kt_c[:, s0 : s0 + 512], qkt_c[64:128, s0 : s0 + 512]
                    )

        def emit_proj_v(t, mm):
            s0 = 512 * t
            hs8 = hst8[:, :, s0 : s0 + 512]
            hs8l = hst8l[:, :, s0 : s0 + 512]
            m = 4 * t + mm
            pv_ = prj_p.tile([128, 512], F32, space="PSUM", tag="prj")
            terms = ((hs8, wv_sb), (hs8l, wv_sb), (hs8, wvl_sb))
            for ti, (ht, wt) in enumerate(terms):
                for c in range(0, 6, 2):
                    nc.tensor.matmul(
                        pv_[:, 0:192],
                        (ht[:, c : c + 2, 128 * mm : 128 * mm + 128]),
                        (wt[:, c : c + 2, :]),
                        start=(ti == 0 and c == 0),
                        stop=(ti == 2 and c == 4 and not has_bias),
                        perf_mode=DR,
                    )
            if has_bias:
                nc.tensor.matmul(
                    pv_[:, 0:192],
                    (ones_sb[0:1, 0:128]),
                    (bv_sb[0:1, :]),
                    start=False,
                    stop=True,
                )
            dst = vall[:, m, :].rearrange("p (h x) -> p h x", h=3)[:, :, 0:64]
            src = pv_[:, 0:192].rearrange("p (h x) -> p h x", h=3)
            nc.vector.tensor_copy(dst, src)

        blk_state = {}

        def emit_scores(n):
            pres = [i for i in range(5) if 0 <= n - 2 + i < NKC]
            lo, hi = pres[0], pres[-1]
            q0 = QB * n
            sab = sab_p.tile([128, 2, 640], F32, space="PSUM", tag="sab")
            sc = sc_p.tile([128, 640], F32, space="PSUM", tag="sc")
            for i in pres:
                m = n - 2 + i
                nc.tensor.matmul(
                    sab[:, 0, 128 * i : 128 * i + 128],
                    (kt_ab[0:64, 128 * m : 128 * m + 128]),
                    (qt_ab[0:64, q0 : q0 + 128]),
                    start=True,
                    stop=True,
                    tile_position=(0, 0),
                )
                nc.tensor.matmul(
                    sab[:, 1, 128 * i : 128 * i + 128],
                    (kt_ab[64:128, 128 * m : 128 * m + 128]),
                    (qt_ab[64:128, q0 : q0 + 128]),
                    start=True,
                    stop=True,
                    tile_position=(64, 0),
                )
            for i in pres:
                m = n - 2 + i
                nc.tensor.matmul(
                    sc[:, 128 * i : 128 * i + 128],
                    (kt_c[:, 128 * m : 128 * m + 128]),
                    (qkt_c[0:64, q0 : q0 + 128]),
                    start=True,
                    stop=True,
                    tile_position=(0, 0),
                )
            # exp (Act) -> bf16 probs in SBUF, then band masks (DVE)
            pt = pt_p.tile([128, 3, 640], BF16, tag="pt")
            c0, c1 = 128 * lo, 128 * hi + 128
            dsc = 1.0 / (QKS * QKS * 8.0)
            nc.scalar.activation(pt[:, 0:2, c0:c1], sab[:, :, c0:c1], EXP, scale=dsc)
            nc.scalar.activation(
                pt[:, 2:3, c0:c1],
                sc[:].rearrange("p (o c) -> p o c", o=1)[:, :, c0:c1],
                EXP,
                scale=dsc,
            )
            ptv = pt[:].rearrange("p h (i c) -> p h i c", c=128)
            if n >= 2 and n <= NQB - 3:
                ptm = ptv[:, :, 0:5:4, :]
                nc.vector.tensor_mul(ptm, ptm, m3_sb[:].rearrange("p s h c -> p h s c"))
            elif n >= 2:
                ptm = ptv[:, :, 0, :]
                nc.vector.tensor_mul(ptm, ptm, m3_sb[:, 0, :, :])
            elif n <= NQB - 3:
                ptm = ptv[:, :, 4, :]
                nc.vector.tensor_mul(ptm, ptm, m3_sb[:, 1, :, :])
            if has_kmask:
                for i in pres:
                    m = n - 2 + i
                    nc.vector.tensor_scalar_mul(
                        pt[:, :, 128 * i : 128 * i + 128],
                        pt[:, :, 128 * i : 128 * i + 128],
                        kpad_sb[:, m : m + 1],
                    )
            blk_state[n] = (pt, pres)

        def emit_pv(n, last=False):
            pt, pres = blk_state.pop(n)
            q0 = QB * n
            pvt = prj_p.tile([128, 512], F32, space="PSUM", tag="prj")
            pv = pvt[:, 0:195]
            for h in range(3):
                for oi, i in enumerate(pres):
                    m = n - 2 + i
                    nc.tensor.matmul(
                        pv[:, 65 * h : 65 * h + 65],
                        (pt[:, h, 128 * i : 128 * i + 128]),
                        (vall[:, m, 65 * h : 65 * h + 65]),
                        start=(oi == 0),
                        stop=(oi == len(pres) - 1),
                    )
            osb = wk_p.tile([128, 195], F32, name="osb")
            nc.vector.tensor_copy(osb[:], pv[:])
            nc.sync.dma_start(out_d[q0 : q0 + 128, 0:195], osb[:])

        # Schedule: blocks processed in descending order so proj pieces
        # (qk of tile t first needed by block 4t+5, v chunk m by block m+2)
        # trickle down to block 2, keeping PE fed in the Act-bound stretch;
        # the last blocks (1, 0) are the short boundary blocks, minimizing
        # the final drain. PV of the previous block is emitted after the
        # next block's scores so the exp/mask latency chain pipelines.
        done_kt = set()
        done_q = set()
        done_v = set()

        deferred = [False]

        def need(n):
            # j1 (kt_ab) + j2 (q/kt_c) are needed with the key chunks
            # (first use at block 4t+5 descending); j0 (qt_ab) only with the
            # query block (first use at 4t+3) - later deadline smooths lumps
            qk_js = []
            for t in range((n + 2) // 4, max((n - 2) // 4, 0) - 1, -1):
                if t < NT and t not in done_kt:
                    done_kt.add(t)
                    qk_js += [(t, 1), (t, 2)]
            t0 = max(n - 1, 0) // 4
            if t0 not in done_q:
                done_q.add(t0)
                qk_js.append((t0, 0))
            vs = []
            for m in range(min(n, NKC - 1), max(n - 3, -1), -1):
                if m not in done_v:
                    done_v.add(m)
                    vs.append(m)
            for t, j in qk_js:
                emit_proj_qk_j(t, j)
            if not deferred[0]:
                deferred[0] = True
                deferred_const_dmas()
            for m in vs:
                emit_proj_v(m // 4, m % 4)

        prevs = []
        for n in range(NQB - 1, -1, -1):
            if n >= 9 and (n - 9) % 4 == 0 and (n - 9) // 4 <= NT - 3:
                dma_hst_tile((n - 9) // 4)
            need(n)
            if len(prevs) == 9:
                emit_pv(prevs.pop(0), last=False)
            emit_scores(n)
            prevs.append(n)
        for i, p in enumerate(prevs):
            emit_pv(p, last=(i >= len(prevs) - 2))
    nc.compile()
    return nc


_prog_cache = {}


def _get_program(has_bias, has_kmask):
    key = (has_bias, has_kmask)
    if key not in _prog_cache:
        _prog_cache[key] = build_program(has_bias, has_kmask)
    return _prog_cache[key]


def _band_masks():
    """Multiplicative band masks [128, 2*3*128] bf16: side-major, head-
    replicated. side 0 masks key chunk n-2 (keep qq <= kk); side 1 masks
    chunk n+2 (keep kk <= qq)."""
    r = np.arange(128)[:, None]
    q = np.arange(128)[None, :]
    m0 = (q <= r).astype(np.float32)
    m4 = (r <= q).astype(np.float32)
    return np.concatenate([m0, m0, m0, m4, m4, m4], axis=1)


def kernel(hidden_states, attention_mask, Wq, bq, Wk, bk, Wv, bv, _res=[None]):
    import ml_dtypes

    bf16 = ml_dtypes.bfloat16
    f8 = ml_dtypes.float8_e4m3
    hidden_states = np.asarray(hidden_states, np.float32)
    attention_mask = np.asarray(attention_mask, np.float32)
    Wq, Wk, Wv = (np.asarray(w, np.float32) for w in (Wq, Wk, Wv))
    bq, bk, bv = (np.asarray(b_, np.float32) for b_ in (bq, bk, bv))

    scale = 1.0 / np.sqrt(DH)
    has_bias = bool(np.any(bq) or np.any(bk) or np.any(bv))
    has_kmask = bool(np.any(attention_mask < 0))

    hsT32 = [np.ascontiguousarray(hidden_states[b].T) for b in range(B)]
    hs8_hi = [h.astype(f8) for h in hsT32]
    hs8_lo = [(h - h8.astype(np.float32)).astype(f8) for h, h8 in zip(hsT32, hs8_hi)]
    masks = _band_masks().astype(bf16)
    masked = attention_mask < 0  # [B, S]

    in_maps = []
    for core in range(N_CORES):
        b, h0 = core // 4, (core % 4) * HPC
        sl = slice(h0 * DH, (h0 + HPC) * DH)
        wq = Wq[:, sl] * 64.0
        wk = Wk[:, sl] * 64.0
        wqk = np.concatenate(
            [wq[:, 0:128], wk[:, 0:128], wq[:, 128:192], wk[:, 128:192]], axis=1
        )
        wqk_hi = wqk.astype(f8)
        wqk_lo = (wqk - wqk_hi.astype(np.float32)).astype(f8)
        wv64 = np.ascontiguousarray(Wv[:, sl]) * 64.0
        wv_hi = wv64.astype(f8)
        wv_lo = (wv64 - wv_hi.astype(np.float32)).astype(f8)
        m = {
            "hs8": hs8_hi[b],
            "hs8l": hs8_lo[b],
            "wqk8": np.ascontiguousarray(wqk_hi),
            "wqk8l": np.ascontiguousarray(wqk_lo),
            "wv8": wv_hi,
            "wv8l": wv_lo,
            "mask3": masks,
        }
        if has_bias:
            bq_s = bq[sl] * 64.0
            bk_s = bk[sl] * 64.0
            m["bqk"] = np.concatenate(
                [bq_s[0:128], bk_s[0:128], bq_s[128:192], bk_s[128:192]]
            ).reshape(1, 384).astype(bf16)
            m["bv"] = (bv[sl] * 64.0).reshape(1, 192).astype(bf16)
        if has_kmask:
            keep = (~masked[b]).astype(np.float32).reshape(NKC, 128).T
            m["kpad"] = np.ascontiguousarray(keep)
        in_maps.append(m)

    nc = _get_program(has_bias, has_kmask)
    res = run_bass_kernel_spmd(nc, in_maps, list(range(N_CORES)))
    _res[0] = res

    out = np.empty((B, S, D), np.float32)
    for core in range(N_CORES):
        b, h0 = core // 4, (core % 4) * HPC
        r = res.results[core]["out"].reshape(S, 3, 65)
        o = r[:, :, 0:64] / r[:, :, 64:65]
        if has_kmask:
            o = np.where(masked[b][:, None, None], 0.0, o)
        out[b, :, h0 * DH : (h0 + HPC) * DH] = o.reshape(S, 192)
    return out
